# revision 23
# baseline (speedup 1.0000x reference)
"""Trainium2 Bass kernel for nn_Loss_factory_12429635355015.

Loss = NLLSurv + CohortLoss(intra + inter) over a [4, 8192, 4, 256] cohort bank.

Strategy (memory-bound, 8 NeuronCores):
  - Shard cohort_bank along the N (bank-entry) axis: each core streams its
    16 MiB shard once at HBM line rate (8 tiles x 2 MiB contiguous DMAs).
  - Per 512-entry tile (4 entries per partition):
      DVE:  component-sum over the 4 bank components  (S = sum_j bank[:,j,:])
      ACT:  Square+accum_out -> per-entry sum-of-squares; rsqrt via exp(-.5 ln x)
      PE :  2x transpose S -> [c, n] chunks; matmul vs l2-normalized anchors
      ACT:  e = exp(sims * (0.5/||S||))   (per-partition scale fuses l2norm+tau)
      PE :  ones-matmul accumulates per-class per-sample sums in PSUM
  - NLL + intra terms are computed on-device from host-encoded one-hots
    (index encoding only; all arithmetic on device).
  - Each core outputs [ep_partial, en_partial, nll+intra]; the host sums the
    two scalars across cores (the 'all-reduce two scalars' step) and applies
    the final -log((ep+eps)/(ep+en+eps)).
"""

import math
import os
import sys

import numpy as np

for _p in ("/opt/trn_rl_repo",):
    if _p not in sys.path and os.path.isdir(_p):
        sys.path.insert(0, _p)

import concourse.bacc as bacc
import concourse.tile as tile
from concourse import mybir
from concourse.bass_utils import run_bass_kernel_spmd

# Pin every activation to the one table set that contains all functions this
# kernel uses (Square/Ln/Exp/Copy/Abs/Identity). Without this, Bacc's
# first-match set selection alternates between sets (Ln lives outside the
# default exp set) and reloads the ACT tables ~1.3us per switch every tile.
_ACT_SET = "natural_log_exp_and_others"


def _pin_act_tables():
    import functools
    import concourse.hw_specs as hw_specs
    if getattr(hw_specs.get_activation_tables, "_pinned", False):
        return
    orig = hw_specs.get_activation_tables

    @functools.cache
    def pinned(arch):
        tabs = orig(arch)
        return {k: (v if k == _ACT_SET else set()) for k, v in tabs.items()}

    pinned._pinned = True
    hw_specs.get_activation_tables = pinned
    bacc.get_activation_tables = pinned


_pin_act_tables()

F32 = mybir.dt.float32
AF = mybir.ActivationFunctionType

# Problem constants (hardcoded per spec).
B = 64            # batch
K = 4             # n_cls
C = 256           # feature dim
NB = 8192         # bank entries per class (global)
NCORES = 8
NSH = NB // NCORES          # 1024 bank entries per class per core
ROWS = K * NSH              # 4096 rows of [4*256] per core
NT = 512                    # bank entries per tile (2 MiB)
TILES = ROWS // NT          # 8
TILES_PER_CLASS = NSH // NT # 2
EPG = NT // 128             # 4 entries per partition per tile
LN_HALF = math.log(0.5)
EPS_NLL = 1e-7
EPS_COH = 1e-8


def _build():
    nc = bacc.Bacc("TRN2", target_bir_lowering=False, debug=False,
                   enable_asserts=False, num_devices=NCORES)

    bank = nc.dram_tensor("bank", [ROWS, 1024], F32, kind="ExternalInput")
    # All small per-sample tensors ride in one [B, SMW] tensor -> ONE DMA at
    # startup instead of nine serialized ~620ns HWDGE launches.
    # Layout: indiv(1024) | gp(512) | haz(4) | spad(5) | ohy(5) | ohy1(5)
    #         | oh4(4) | cfs(2)
    SMW = 1024 + 512 + K + (K + 1) * 3 + K + 2
    smallin = nc.dram_tensor("smallin", [B, SMW], F32, kind="ExternalInput")

    out_d = nc.dram_tensor("out_vec", [4, 1], F32, kind="ExternalOutput")

    import ml_dtypes
    ident_d = nc.inline_tensor(np.eye(128, dtype=np.float32), "ident")
    ident_bf_d = nc.inline_tensor(np.eye(128, dtype=ml_dtypes.bfloat16), "ident_bf")
    ones_d = nc.inline_tensor(np.ones((128, 1), dtype=np.float32), "ones_col")

    v = nc.vector
    a = nc.scalar

    with tile.TileContext(nc) as tc:
        from contextlib import ExitStack
        with ExitStack() as ctx:
            const = ctx.enter_context(tc.tile_pool(name="const", bufs=1))
            small = ctx.enter_context(tc.tile_pool(name="small", bufs=1))
            tpool = ctx.enter_context(tc.tile_pool(name="T", bufs=8))
            spool = ctx.enter_context(tc.tile_pool(name="S", bufs=6))
            stpool = ctx.enter_context(tc.tile_pool(name="STsb", bufs=8))
            epool = ctx.enter_context(tc.tile_pool(name="esb", bufs=6))
            sqpool = ctx.enter_context(tc.tile_pool(name="sq", bufs=4))
            ps_st = ctx.enter_context(tc.tile_pool(name="ps_st", bufs=4, space="PSUM"))
            ps_p = ctx.enter_context(tc.tile_pool(name="ps_p", bufs=2, space="PSUM"))
            ps_one = ctx.enter_context(tc.tile_pool(name="ps_one", bufs=1, space="PSUM"))

            BF16 = mybir.dt.bfloat16
            # Issue the small-input DMA FIRST: once the bank stream saturates
            # HBM, a q1 transfer packet-interleaves with it and takes ~15us —
            # gating the whole DVE program (anchors come first in its stream).
            sm_sb = small.tile([B, SMW], F32)
            nc.sync.dma_start(out=sm_sb[:], in_=smallin[:])
            ident_sb = const.tile([128, 128], F32)
            nc.sync.dma_start(out=ident_sb[:], in_=ident_d[:])
            ident_bf = const.tile([128, 128], BF16)
            nc.sync.dma_start(out=ident_bf[:], in_=ident_bf_d[:])
            ones_sb = const.tile([128, 1], F32)
            nc.sync.dma_start(out=ones_sb[:], in_=ones_d[:])

            # ---------- anchors: A = l2norm(mean_j indiv[b,j,:]) ----------
            _o = [0]

            def _sl(w):
                s = _o[0]
                _o[0] += w
                return sm_sb[:, s:s + w]

            ind_sb = _sl(1024)
            gp_sb = _sl(512)
            haz_sb = _sl(K)
            spad_sb = _sl(K + 1)
            ohy_sb = _sl(K + 1)
            ohy1_sb = _sl(K + 1)
            oh4_sb = _sl(K)
            cfs_sb = _sl(2)
            iv = ind_sb.rearrange("p (j c) -> p j c", j=4)
            asum = small.tile([B, C], F32)
            atmp = small.tile([B, C], F32)
            v.tensor_add(asum[:], iv[:, 0, :], iv[:, 1, :])
            v.tensor_add(atmp[:], iv[:, 2, :], iv[:, 3, :])
            v.tensor_add(asum[:], asum[:], atmp[:])
            sqa = small.tile([B, C], F32)
            ssa = small.tile([B, 1], F32)
            a.activation(sqa[:], asum[:], AF.Square, accum_out=ssa[:])
            lna = small.tile([B, 1], F32)
            a.activation(lna[:], ssa[:], AF.Ln)
            rsa = small.tile([B, 1], F32)
            a.activation(rsa[:], lna[:], AF.Exp, scale=-0.5)
            v.tensor_scalar_mul(asum[:], asum[:], rsa[:])
            at_ps = ps_one.tile([128, 2, B], F32, tag="at")
            for h in range(2):
                nc.tensor.transpose(at_ps[:, h, :], asum[:, h * 128:(h + 1) * 128],
                                    ident_sb[0:B, 0:B])
            at_sb = const.tile([128, 2, B], BF16)
            a.copy(at_sb[:], at_ps[:])

            # ---------- side computation (NLL + intra), STT-fused ----------
            # Emitted in chunks BETWEEN tile emissions so the ~8us of small
            # DVE/ACT ops fill per-tile pipeline slack instead of extending
            # the head (pre-loop) or the drain (post-loop).
            BYP = mybir.AluOpType.bypass
            MULOP = mybir.AluOpType.mult
            sN = {}

            def _nll():
                t5 = small.tile([B, K + 1], F32)
                sy = small.tile([B, 1], F32)
                hy = small.tile([B, 1], F32)
                sy1 = small.tile([B, 1], F32)
                v.scalar_tensor_tensor(out=t5[:], in0=spad_sb[:], scalar=0.0,
                                       in1=ohy_sb[:], op0=BYP, op1=MULOP,
                                       accum_out=sy[:])
                v.scalar_tensor_tensor(out=t5[:, 0:K], in0=haz_sb[:], scalar=0.0,
                                       in1=ohy_sb[:, 0:K], op0=BYP, op1=MULOP,
                                       accum_out=hy[:])
                v.scalar_tensor_tensor(out=t5[:], in0=spad_sb[:], scalar=0.0,
                                       in1=ohy1_sb[:], op0=BYP, op1=MULOP,
                                       accum_out=sy1[:])
                for x in (sy, hy, sy1):
                    v.tensor_scalar_max(x[:], x[:], EPS_NLL)
                lsy = small.tile([B, 1], F32)
                lhy = small.tile([B, 1], F32)
                lsy1 = small.tile([B, 1], F32)
                a.activation(lsy[:], sy[:], AF.Ln)
                a.activation(lhy[:], hy[:], AF.Ln)
                a.activation(lsy1[:], sy1[:], AF.Ln)
                tu = small.tile([B, 1], F32)
                tcen = small.tile([B, 1], F32)
                negl = small.tile([B, 1], F32)
                v.tensor_add(tu[:], lsy[:], lhy[:])
                v.tensor_mul(tu[:], tu[:], cfs_sb[:, 1:2])      # *(1-cf)
                v.tensor_mul(tcen[:], lsy1[:], cfs_sb[:, 0:1])  # *cf
                v.tensor_add(negl[:], tu[:], tcen[:])           # = -neg_l per b
                sN["negl"] = negl

            def _intra_sos():
                ssqi = small.tile([B, 4], F32)
                ssqg = small.tile([B, 2], F32)
                prod = small.tile([B, C], F32, tag="iprod")
                for p in range(4):
                    v.scalar_tensor_tensor(
                        out=prod[:], in0=ind_sb[:, p * C:(p + 1) * C], scalar=0.0,
                        in1=ind_sb[:, p * C:(p + 1) * C], op0=BYP, op1=MULOP,
                        accum_out=ssqi[:, p:p + 1])
                for g in range(2):
                    v.scalar_tensor_tensor(
                        out=prod[:], in0=gp_sb[:, g * C:(g + 1) * C], scalar=0.0,
                        in1=gp_sb[:, g * C:(g + 1) * C], op0=BYP, op1=MULOP,
                        accum_out=ssqg[:, g:g + 1])
                rsi = small.tile([B, 4], F32)
                a.activation(rsi[:], ssqi[:], AF.Ln)
                a.activation(rsi[:], rsi[:], AF.Exp, scale=-0.5)
                rsg = small.tile([B, 2], F32)
                a.activation(rsg[:], ssqg[:], AF.Ln)
                a.activation(rsg[:], rsg[:], AF.Exp, scale=-0.5)
                sN["rsi"], sN["rsg"] = rsi, rsg

            def _intra_sims():
                # raw dots on the UN-normalized rows; the l2norm scales are
                # applied afterwards on the tiny [B,8] result (sim_hat =
                # raw_dot * rsi_p * rsg_g, and rsi/rsg > 0 commute with
                # abs/neg) -- saves six [B,256] normalize muls
                D = small.tile([B, 8], F32)
                prod = small.tile([B, C], F32, tag="iprod2")
                for p in range(4):
                    for g in range(2):
                        col = p * 2 + g
                        v.scalar_tensor_tensor(
                            out=prod[:], in0=ind_sb[:, p * C:(p + 1) * C],
                            scalar=0.0, in1=gp_sb[:, g * C:(g + 1) * C],
                            op0=BYP, op1=MULOP, accum_out=D[:, col:col + 1])
                sN["D"] = D

            def _intra_fin():
                Draw = sN["D"]
                rprod = small.tile([B, 4, 2], F32)
                v.tensor_mul(rprod[:],
                             sN["rsi"][:, :, None].broadcast_to([B, 4, 2]),
                             sN["rsg"][:, None, :].broadcast_to([B, 4, 2]))
                D = small.tile([B, 8], F32, tag="Dh")
                v.tensor_mul(D[:], Draw[:], rprod.rearrange("p a b -> p (a b)"))
                U = small.tile([B, 8], F32)
                a.activation(U[:], D[:], AF.Abs)
                # mask==1 entries (cols 0,1,4,7) use -sim instead of |sim|
                v.tensor_scalar_mul(U[:, 0:2], D[:, 0:2], -1.0)
                v.tensor_scalar_mul(U[:, 4:5], D[:, 4:5], -1.0)
                v.tensor_scalar_mul(U[:, 7:8], D[:, 7:8], -1.0)
                isum = small.tile([B, 1], F32)
                v.reduce_sum(isum[:], U[:], axis=mybir.AxisListType.X)
                # contrib_b = -negl/B + isum/(8B) + 1/B -> sums to nll+intra
                c1 = small.tile([B, 1], F32)
                c2 = small.tile([B, 1], F32)
                contrib = small.tile([B, 1], F32)
                v.tensor_scalar_mul(c1[:], sN["negl"][:], -1.0 / B)
                v.tensor_scalar_mul(c2[:], isum[:], 1.0 / (8 * B))
                v.tensor_add(contrib[:], c1[:], c2[:])
                v.tensor_scalar_add(contrib[:], contrib[:], 1.0 / B)
                sN["contrib"] = contrib

            side = {1: _nll, 2: _intra_sos, 3: _intra_sims, 4: _intra_fin}

            # ---------- main loop over bank tiles ----------
            # Per-tile exp-sums land in et_all columns straight from the ACT
            # accumulator; the epilogue folds tile pairs into per-class sums.
            et_all = small.tile([B, TILES], F32)
            for t in range(TILES):
                T_sb = tpool.tile([128, 4096], BF16)
                src = bank[t * NT:(t + 1) * NT, :].rearrange("(p e) x -> p e x", e=EPG)
                # SWDGE cast-DMA: f32 HBM -> bf16 SBUF at line rate
                nc.gpsimd.dma_start(out=T_sb.rearrange("p (e x) -> p e x", e=EPG),
                                    in_=src)
                Tv = T_sb.rearrange("p (e j c) -> p e j c", e=EPG, j=4)
                S_sb = spool.tile([128, 1024], BF16)
                Sv = S_sb.rearrange("p (e c) -> p e c", e=EPG)
                tmp = spool.tile([128, 1024], BF16, tag="tmp")
                tv = tmp.rearrange("p (e c) -> p e c", e=EPG)
                v.tensor_add(Sv[:], Tv[:, :, 0, :], Tv[:, :, 1, :])
                v.tensor_add(tv[:], Tv[:, :, 2, :], Tv[:, :, 3, :])
                v.tensor_add(Sv[:], Sv[:], tv[:])
                # per-entry sum-of-squares, split across ACT (Square+accum)
                # and DVE (fused square+row-sum) to balance the two engines
                ssum4 = spool.tile([128, EPG], F32, tag="ssum")
                sqscr = sqpool.tile([128, C], BF16)
                sqscrf = sqpool.tile([128, C], F32, tag="sqf")
                for e in range(2):
                    a.activation(sqscrf[:], Sv[:, e, :], AF.Square,
                                 accum_out=ssum4[:, e:e + 1])
                for e in range(2, EPG):
                    v.scalar_tensor_tensor(
                        out=sqscr[:], in0=Sv[:, e, :], scalar=0.0,
                        in1=Sv[:, e, :], op0=BYP, op1=MULOP,
                        accum_out=ssum4[:, e:e + 1])
                lh4 = spool.tile([128, EPG], F32, tag="lh4")
                a.activation(lh4[:], ssum4[:], AF.Ln)
                # rsqrt lands in bf16 so the fused normalize runs in 2x mode
                rh4 = spool.tile([128, EPG], BF16, tag="rh4")
                a.activation(rh4[:], lh4[:], AF.Exp, scale=-0.5)
                # normalize S rows in place: ONE broadcast mul over all 4
                # e-groups (per-entry scale broadcast along c)
                v.tensor_mul(Sv[:], Sv[:],
                             rh4[:, :, None].broadcast_to([128, EPG, C]))
                # transpose all 4 e-groups into [c, n=512] chunks (h = c-half)
                st_ps = [ps_st.tile([128, 512], BF16, name="stps", tag="stps")
                         for _ in range(2)]
                for e in range(EPG):
                    for h in range(2):
                        nc.tensor.transpose(
                            st_ps[h][:, e * 128:(e + 1) * 128],
                            S_sb[:, e * C + h * 128: e * C + (h + 1) * 128],
                            ident_bf[:])
                p_ps = ps_p.tile([B, 512], F32)
                for h in range(2):
                    st_sb = stpool.tile([128, 512], BF16)
                    if h == 0:
                        a.copy(st_sb[:], st_ps[h][:])       # ACT has slack
                    else:
                        v.tensor_copy(st_sb[:], st_ps[h][:])
                    nc.tensor.matmul(p_ps[:], at_sb[:, h, :], st_sb[:],
                                     start=(h == 0), stop=(h == 1))
                e_sb = epool.tile([B, 512], BF16)
                a.activation(e_sb[:], p_ps[:], AF.Exp, scale=0.5,
                             accum_out=et_all[:, t:t + 1])
                if t in side:
                    side[t]()

            # ---------- epilogue: partial scalars ----------
            contrib = sN["contrib"]
            E_sb = small.tile([B, K], F32)
            eav = et_all.rearrange("p (k two) -> p k two", two=TILES_PER_CLASS)
            v.tensor_add(E_sb[:], eav[:, :, 0], eav[:, :, 1])
            t4b = small.tile([B, K], F32)
            epb = small.tile([B, 1], F32)
            rsum = small.tile([B, 1], F32)
            enb = small.tile([B, 1], F32)
            v.tensor_mul(t4b[:], E_sb[:], oh4_sb[:])
            v.reduce_sum(epb[:], t4b[:], axis=mybir.AxisListType.X)
            v.reduce_sum(rsum[:], E_sb[:], axis=mybir.AxisListType.X)
            v.tensor_scalar_mul(enb[:], epb[:], -1.0)
            v.tensor_add(enb[:], enb[:], rsum[:])
            F = small.tile([B, 4], F32)
            v.memset(F[:], 0.0)
            v.tensor_scalar_mul(F[:, 0:1], epb[:], 1.0 / (B * NB))
            v.tensor_scalar_mul(F[:, 1:2], enb[:], 1.0 / (B * (K - 1) * NB))
            v.tensor_copy(F[:, 2:3], contrib[:])
            out_ps = ps_one.tile([4, 1], F32, tag="o3")
            nc.tensor.matmul(out_ps[:], F[:], ones_sb[0:B, :], start=True, stop=True)
            out_sb = small.tile([4, 1], F32)
            a.copy(out_sb[:], out_ps[:])
            nc.sync.dma_start(out=out_d[:], in_=out_sb[:])

    nc.compile()
    return nc


_NC = None


def _get_nc():
    global _NC
    if _NC is None:
        _NC = _build()
    return _NC


def _make_in_maps(hazards, S, indiv, gene, path, cohort_bank, label, c):
    hazards = np.asarray(hazards, dtype=np.float32)
    S = np.asarray(S, dtype=np.float32)
    indiv = np.asarray(indiv, dtype=np.float32)
    gene = np.asarray(gene, dtype=np.float32)
    path = np.asarray(path, dtype=np.float32)
    cohort_bank = np.asarray(cohort_bank, dtype=np.float32)
    label = np.asarray(label)
    c = np.asarray(c)

    oh5 = np.zeros((B, K + 1), np.float32)
    oh5[np.arange(B), label] = 1.0
    oh5b = np.zeros((B, K + 1), np.float32)
    oh5b[np.arange(B), label + 1] = 1.0
    oh4 = oh5[:, :K].copy()
    spad = np.concatenate([np.ones((B, 1), np.float32), S], axis=1)
    cfs = np.stack([c.astype(np.float32), 1.0 - c.astype(np.float32)], axis=1)
    smallin = np.ascontiguousarray(np.concatenate([
        indiv.reshape(B, -1),
        gene.reshape(B, -1), path.reshape(B, -1),
        hazards, spad, oh5, oh5b, oh4, cfs,
    ], axis=1, dtype=np.float32))
    common = dict(smallin=smallin)
    bankf = cohort_bank.reshape(K, NB, 1024)
    in_maps = []
    for i in range(NCORES):
        shard = np.ascontiguousarray(
            bankf[:, i * NSH:(i + 1) * NSH, :]).reshape(ROWS, 1024)
        in_maps.append({**common, "bank": shard})
    return in_maps


_LAST_RESULTS = None  # stashed for test.py introspection


def kernel(hazards, S, indiv, gene, path, cohort_bank, label, c):
    global _LAST_RESULTS
    os.environ.setdefault("NEURON_RT_RESET_CORES", "1")
    nc = _get_nc()
    in_maps = _make_in_maps(hazards, S, indiv, gene, path, cohort_bank, label, c)
    trace = bool(int(os.environ.get("TRNK_TRACE", "0")))
    res = run_bass_kernel_spmd(nc, in_maps, core_ids=list(range(NCORES)),
                               trace=trace)
    _LAST_RESULTS = res
    outs = np.stack([r["out_vec"][:, 0] for r in res.results])  # [8, 4]
    ep = float(outs[:, 0].sum())
    en = float(outs[:, 1].sum())
    other = float(outs[:, 2].mean())
    loss = other - math.log((ep + EPS_COH) / (ep + en + EPS_COH))
    return np.float32(loss)



# revision 27
# speedup vs baseline: 1.0319x; 1.0319x over previous
"""Trainium2 Bass kernel for nn_Loss_factory_12429635355015.

Loss = NLLSurv + CohortLoss(intra + inter) over a [4, 8192, 4, 256] cohort bank.

Strategy (memory-bound, 8 NeuronCores):
  - Shard cohort_bank along the N (bank-entry) axis: each core streams its
    16 MiB shard once at HBM line rate (8 tiles x 2 MiB SWDGE cast-DMAs,
    f32 HBM -> bf16 SBUF); the small [B,*] tensors are replicated and ride
    in ONE merged DMA issued before the bank stream starts.
  - Per 512-entry tile (4 entries per partition), balanced across engines:
      DVE:  component-sum (3 bf16 adds); half the per-entry sum-of-squares
            (fused scalar_tensor_tensor square+row-sum); ONE broadcast mul
            normalizes all 4 e-groups; one PSUM->SBUF copy
      ACT:  other half of the sum-of-squares (Square+accum_out); rsqrt via
            exp(-.5 ln x); one PSUM->SBUF copy; final e = exp(.5 * sims)
            with accum_out delivering the per-tile row sums for free
      PE :  8x [128,128] transpose S -> [c, n] chunks; 2 matmuls vs
            l2-normalized anchors
  - NLL + intra terms are computed on-device from host-encoded one-hots
    (index encoding only; all arithmetic on device) and their instruction
    emission is interleaved between tile emissions so they fill pipeline
    slack instead of extending the head or drain. Intra sims use raw dots
    scaled at the end by rsi_p*rsg_g on [B,8] (norm scales commute with
    abs/neg), avoiding six [B,256] normalize muls.
  - Each core outputs [ep_partial, en_partial, nll+intra]; the host sums the
    two scalars across cores (the 'all-reduce two scalars' step) and applies
    the final -log((ep+eps)/(ep+en+eps)).
"""

import math
import os
import sys

import numpy as np

for _p in ("/opt/trn_rl_repo",):
    if _p not in sys.path and os.path.isdir(_p):
        sys.path.insert(0, _p)

import concourse.bacc as bacc
import concourse.tile as tile
from concourse import mybir
from concourse.bass_utils import run_bass_kernel_spmd

# Pin every activation to the one table set that contains all functions this
# kernel uses (Square/Ln/Exp/Copy/Abs/Identity). Without this, Bacc's
# first-match set selection alternates between sets (Ln lives outside the
# default exp set) and reloads the ACT tables ~1.3us per switch every tile.
_ACT_SET = "natural_log_exp_and_others"


def _pin_act_tables():
    import functools
    import concourse.hw_specs as hw_specs
    if getattr(hw_specs.get_activation_tables, "_pinned", False):
        return
    orig = hw_specs.get_activation_tables

    @functools.cache
    def pinned(arch):
        tabs = orig(arch)
        return {k: (v if k == _ACT_SET else set()) for k, v in tabs.items()}

    pinned._pinned = True
    hw_specs.get_activation_tables = pinned
    bacc.get_activation_tables = pinned


_pin_act_tables()

F32 = mybir.dt.float32
AF = mybir.ActivationFunctionType

# Problem constants (hardcoded per spec).
B = 64            # batch
K = 4             # n_cls
C = 256           # feature dim
NB = 8192         # bank entries per class (global)
NCORES = 8
NSH = NB // NCORES          # 1024 bank entries per class per core
ROWS = K * NSH              # 4096 rows of [4*256] per core
NT = 512                    # bank entries per tile (2 MiB)
TILES = ROWS // NT          # 8
TILES_PER_CLASS = NSH // NT # 2
EPG = NT // 128             # 4 entries per partition per tile
LN_HALF = math.log(0.5)
EPS_NLL = 1e-7
EPS_COH = 1e-8


def _build():
    nc = bacc.Bacc("TRN2", target_bir_lowering=False, debug=False,
                   enable_asserts=False, num_devices=NCORES)

    bank = nc.dram_tensor("bank", [ROWS, 1024], F32, kind="ExternalInput")
    # All small per-sample tensors ride in one [B, SMW] tensor -> ONE DMA at
    # startup instead of nine serialized ~620ns HWDGE launches.
    # Layout: indiv(1024) | gp(512) | haz(4) | spad(5) | ohy(5) | ohy1(5)
    #         | oh4(4) | cfs(2)
    SMW = 1024 + 512 + K + (K + 1) * 3 + K + 2
    smallin = nc.dram_tensor("smallin", [B, SMW], F32, kind="ExternalInput")

    out_d = nc.dram_tensor("out_vec", [4, 1], F32, kind="ExternalOutput")

    import ml_dtypes
    ident_d = nc.inline_tensor(np.eye(128, dtype=np.float32), "ident")
    ident_bf_d = nc.inline_tensor(np.eye(128, dtype=ml_dtypes.bfloat16), "ident_bf")
    ones_d = nc.inline_tensor(np.ones((128, 1), dtype=np.float32), "ones_col")

    v = nc.vector
    a = nc.scalar

    with tile.TileContext(nc) as tc:
        from contextlib import ExitStack
        with ExitStack() as ctx:
            const = ctx.enter_context(tc.tile_pool(name="const", bufs=1))
            small = ctx.enter_context(tc.tile_pool(name="small", bufs=1))
            tpool = ctx.enter_context(tc.tile_pool(name="T", bufs=6))
            spool = ctx.enter_context(tc.tile_pool(name="S", bufs=4))
            stpool = ctx.enter_context(tc.tile_pool(name="STsb", bufs=6))
            epool = ctx.enter_context(tc.tile_pool(name="esb", bufs=4))
            sqpool = ctx.enter_context(tc.tile_pool(name="sq", bufs=3))
            ps_st = ctx.enter_context(tc.tile_pool(name="ps_st", bufs=4, space="PSUM"))
            ps_p = ctx.enter_context(tc.tile_pool(name="ps_p", bufs=2, space="PSUM"))
            ps_one = ctx.enter_context(tc.tile_pool(name="ps_one", bufs=1, space="PSUM"))

            BF16 = mybir.dt.bfloat16
            # Issue the small-input DMA FIRST: once the bank stream saturates
            # HBM, a q1 transfer packet-interleaves with it and takes ~15us —
            # gating the whole DVE program (anchors come first in its stream).
            sm_sb = small.tile([B, SMW], F32)
            nc.sync.dma_start(out=sm_sb[:], in_=smallin[:])
            ident_sb = const.tile([128, 128], F32)
            nc.sync.dma_start(out=ident_sb[:], in_=ident_d[:])
            ident_bf = const.tile([128, 128], BF16)
            nc.sync.dma_start(out=ident_bf[:], in_=ident_bf_d[:])
            ones_sb = const.tile([128, 1], F32)
            nc.sync.dma_start(out=ones_sb[:], in_=ones_d[:])

            # ---------- anchors: A = l2norm(mean_j indiv[b,j,:]) ----------
            _o = [0]

            def _sl(w):
                s = _o[0]
                _o[0] += w
                return sm_sb[:, s:s + w]

            ind_sb = _sl(1024)
            gp_sb = _sl(512)
            haz_sb = _sl(K)
            spad_sb = _sl(K + 1)
            ohy_sb = _sl(K + 1)
            ohy1_sb = _sl(K + 1)
            oh4_sb = _sl(K)
            cfs_sb = _sl(2)
            iv = ind_sb.rearrange("p (j c) -> p j c", j=4)
            asum = small.tile([B, C], F32)
            atmp = small.tile([B, C], F32)
            v.tensor_add(asum[:], iv[:, 0, :], iv[:, 1, :])
            v.tensor_add(atmp[:], iv[:, 2, :], iv[:, 3, :])
            v.tensor_add(asum[:], asum[:], atmp[:])
            sqa = small.tile([B, C], F32)
            ssa = small.tile([B, 1], F32)
            a.activation(sqa[:], asum[:], AF.Square, accum_out=ssa[:])
            lna = small.tile([B, 1], F32)
            a.activation(lna[:], ssa[:], AF.Ln)
            rsa = small.tile([B, 1], F32)
            a.activation(rsa[:], lna[:], AF.Exp, scale=-0.5)
            v.tensor_scalar_mul(asum[:], asum[:], rsa[:])
            at_ps = ps_one.tile([128, 2, B], F32, tag="at")
            for h in range(2):
                nc.tensor.transpose(at_ps[:, h, :], asum[:, h * 128:(h + 1) * 128],
                                    ident_sb[0:B, 0:B])
            at_sb = const.tile([128, 2, B], BF16)
            a.copy(at_sb[:], at_ps[:])

            # ---------- side computation (NLL + intra), STT-fused ----------
            # Emitted in chunks BETWEEN tile emissions so the ~8us of small
            # DVE/ACT ops fill per-tile pipeline slack instead of extending
            # the head (pre-loop) or the drain (post-loop).
            BYP = mybir.AluOpType.bypass
            MULOP = mybir.AluOpType.mult
            sN = {}

            def _nll():
                t5 = small.tile([B, K + 1], F32)
                sy = small.tile([B, 1], F32)
                hy = small.tile([B, 1], F32)
                sy1 = small.tile([B, 1], F32)
                v.scalar_tensor_tensor(out=t5[:], in0=spad_sb[:], scalar=0.0,
                                       in1=ohy_sb[:], op0=BYP, op1=MULOP,
                                       accum_out=sy[:])
                v.scalar_tensor_tensor(out=t5[:, 0:K], in0=haz_sb[:], scalar=0.0,
                                       in1=ohy_sb[:, 0:K], op0=BYP, op1=MULOP,
                                       accum_out=hy[:])
                v.scalar_tensor_tensor(out=t5[:], in0=spad_sb[:], scalar=0.0,
                                       in1=ohy1_sb[:], op0=BYP, op1=MULOP,
                                       accum_out=sy1[:])
                for x in (sy, hy, sy1):
                    v.tensor_scalar_max(x[:], x[:], EPS_NLL)
                lsy = small.tile([B, 1], F32)
                lhy = small.tile([B, 1], F32)
                lsy1 = small.tile([B, 1], F32)
                a.activation(lsy[:], sy[:], AF.Ln)
                a.activation(lhy[:], hy[:], AF.Ln)
                a.activation(lsy1[:], sy1[:], AF.Ln)
                tu = small.tile([B, 1], F32)
                tcen = small.tile([B, 1], F32)
                negl = small.tile([B, 1], F32)
                v.tensor_add(tu[:], lsy[:], lhy[:])
                v.tensor_mul(tu[:], tu[:], cfs_sb[:, 1:2])      # *(1-cf)
                v.tensor_mul(tcen[:], lsy1[:], cfs_sb[:, 0:1])  # *cf
                v.tensor_add(negl[:], tu[:], tcen[:])           # = -neg_l per b
                sN["negl"] = negl

            def _intra_sos():
                # on ACT (Square+accum): DVE is the pacing engine, ACT idles
                # during the drain
                ssqi = small.tile([B, 4], F32)
                ssqg = small.tile([B, 2], F32)
                prod = small.tile([B, C], F32, tag="iprod")
                for p in range(4):
                    a.activation(prod[:], ind_sb[:, p * C:(p + 1) * C],
                                 AF.Square, accum_out=ssqi[:, p:p + 1])
                for g in range(2):
                    a.activation(prod[:], gp_sb[:, g * C:(g + 1) * C],
                                 AF.Square, accum_out=ssqg[:, g:g + 1])
                rsi = small.tile([B, 4], F32)
                a.activation(rsi[:], ssqi[:], AF.Ln)
                a.activation(rsi[:], rsi[:], AF.Exp, scale=-0.5)
                rsg = small.tile([B, 2], F32)
                a.activation(rsg[:], ssqg[:], AF.Ln)
                a.activation(rsg[:], rsg[:], AF.Exp, scale=-0.5)
                sN["rsi"], sN["rsg"] = rsi, rsg

            def _intra_sims():
                # raw dots on the UN-normalized rows; the l2norm scales are
                # applied afterwards on the tiny [B,8] result (sim_hat =
                # raw_dot * rsi_p * rsg_g, and rsi/rsg > 0 commute with
                # abs/neg) -- saves six [B,256] normalize muls
                D = small.tile([B, 8], F32)
                prod = small.tile([B, C], F32, tag="iprod2")
                for p in range(4):
                    for g in range(2):
                        col = p * 2 + g
                        v.scalar_tensor_tensor(
                            out=prod[:], in0=ind_sb[:, p * C:(p + 1) * C],
                            scalar=0.0, in1=gp_sb[:, g * C:(g + 1) * C],
                            op0=BYP, op1=MULOP, accum_out=D[:, col:col + 1])
                sN["D"] = D

            def _intra_fin():
                Draw = sN["D"]
                rprod = small.tile([B, 4, 2], F32)
                v.tensor_mul(rprod[:],
                             sN["rsi"][:, :, None].broadcast_to([B, 4, 2]),
                             sN["rsg"][:, None, :].broadcast_to([B, 4, 2]))
                D = small.tile([B, 8], F32, tag="Dh")
                v.tensor_mul(D[:], Draw[:], rprod.rearrange("p a b -> p (a b)"))
                U = small.tile([B, 8], F32)
                a.activation(U[:], D[:], AF.Abs)
                # mask==1 entries (cols 0,1,4,7) use -sim instead of |sim|
                v.tensor_scalar_mul(U[:, 0:2], D[:, 0:2], -1.0)
                v.tensor_scalar_mul(U[:, 4:5], D[:, 4:5], -1.0)
                v.tensor_scalar_mul(U[:, 7:8], D[:, 7:8], -1.0)
                isum = small.tile([B, 1], F32)
                v.reduce_sum(isum[:], U[:], axis=mybir.AxisListType.X)
                # contrib_b = -negl/B + isum/(8B) + 1/B -> sums to nll+intra
                c1 = small.tile([B, 1], F32)
                c2 = small.tile([B, 1], F32)
                contrib = small.tile([B, 1], F32)
                v.tensor_scalar_mul(c1[:], sN["negl"][:], -1.0 / B)
                v.tensor_scalar_mul(c2[:], isum[:], 1.0 / (8 * B))
                v.tensor_add(contrib[:], c1[:], c2[:])
                v.tensor_scalar_add(contrib[:], contrib[:], 1.0 / B)
                sN["contrib"] = contrib

            side = {1: _nll, 2: _intra_sos, 3: _intra_sims, 4: _intra_fin}

            # ---------- main loop over bank tiles ----------
            # Per-tile exp-sums land in et_all columns straight from the ACT
            # accumulator; the epilogue folds tile pairs into per-class sums.
            et_all = small.tile([B, TILES], F32)
            for t in range(TILES):
                T_sb = tpool.tile([128, 4096], BF16)
                src = bank[t * NT:(t + 1) * NT, :].rearrange("(p e) x -> p e x", e=EPG)
                # SWDGE cast-DMA: f32 HBM -> bf16 SBUF at line rate
                nc.gpsimd.dma_start(out=T_sb.rearrange("p (e x) -> p e x", e=EPG),
                                    in_=src)
                Tv = T_sb.rearrange("p (e j c) -> p e j c", e=EPG, j=4)
                S_sb = spool.tile([128, 1024], BF16)
                Sv = S_sb.rearrange("p (e c) -> p e c", e=EPG)
                tmp = spool.tile([128, 1024], BF16, tag="tmp")
                tv = tmp.rearrange("p (e c) -> p e c", e=EPG)
                v.tensor_add(Sv[:], Tv[:, :, 0, :], Tv[:, :, 1, :])
                v.tensor_add(tv[:], Tv[:, :, 2, :], Tv[:, :, 3, :])
                v.tensor_add(Sv[:], Sv[:], tv[:])
                # per-entry sum-of-squares, split across ACT (Square+accum)
                # and DVE (fused square+row-sum) to balance the two engines
                ssum4 = spool.tile([128, EPG], F32, tag="ssum")
                sqscr = sqpool.tile([128, C], BF16)
                sqscrf = sqpool.tile([128, C], F32, tag="sqf")
                for e in range(2):
                    a.activation(sqscrf[:], Sv[:, e, :], AF.Square,
                                 accum_out=ssum4[:, e:e + 1])
                for e in range(2, EPG):
                    v.scalar_tensor_tensor(
                        out=sqscr[:], in0=Sv[:, e, :], scalar=0.0,
                        in1=Sv[:, e, :], op0=BYP, op1=MULOP,
                        accum_out=ssum4[:, e:e + 1])
                lh4 = spool.tile([128, EPG], F32, tag="lh4")
                a.activation(lh4[:], ssum4[:], AF.Ln)
                # rsqrt lands in bf16 so the fused normalize runs in 2x mode
                rh4 = spool.tile([128, EPG], BF16, tag="rh4")
                a.activation(rh4[:], lh4[:], AF.Exp, scale=-0.5)
                # normalize S rows in place: ONE broadcast mul over all 4
                # e-groups (per-entry scale broadcast along c)
                v.tensor_mul(Sv[:], Sv[:],
                             rh4[:, :, None].broadcast_to([128, EPG, C]))
                # transpose all 4 e-groups into [c, n=512] chunks (h = c-half),
                # both halves into ONE 1-bank PSUM tile -> ONE ACT copy
                st_ps = ps_st.tile([128, 1024], BF16, name="stps", tag="stps")
                stv = st_ps.rearrange("p (h x) -> p h x", h=2)
                for e in range(EPG):
                    for h in range(2):
                        nc.tensor.transpose(
                            stv[:, h, e * 128:(e + 1) * 128],
                            S_sb[:, e * C + h * 128: e * C + (h + 1) * 128],
                            ident_bf[:])
                st_sb = stpool.tile([128, 1024], BF16)
                a.copy(st_sb[:], st_ps[:])
                sv2 = st_sb.rearrange("p (h x) -> p h x", h=2)
                p_ps = ps_p.tile([B, 512], F32)
                for h in range(2):
                    nc.tensor.matmul(p_ps[:], at_sb[:, h, :], sv2[:, h, :],
                                     start=(h == 0), stop=(h == 1))
                e_sb = epool.tile([B, 512], BF16)
                a.activation(e_sb[:], p_ps[:], AF.Exp, scale=0.5,
                             accum_out=et_all[:, t:t + 1])
                if t in side:
                    side[t]()

            # ---------- epilogue: partial scalars ----------
            contrib = sN["contrib"]
            E_sb = small.tile([B, K], F32)
            eav = et_all.rearrange("p (k two) -> p k two", two=TILES_PER_CLASS)
            v.tensor_add(E_sb[:], eav[:, :, 0], eav[:, :, 1])
            t4b = small.tile([B, K], F32)
            epb = small.tile([B, 1], F32)
            rsum = small.tile([B, 1], F32)
            enb = small.tile([B, 1], F32)
            v.tensor_mul(t4b[:], E_sb[:], oh4_sb[:])
            v.reduce_sum(epb[:], t4b[:], axis=mybir.AxisListType.X)
            v.reduce_sum(rsum[:], E_sb[:], axis=mybir.AxisListType.X)
            v.tensor_scalar_mul(enb[:], epb[:], -1.0)
            v.tensor_add(enb[:], enb[:], rsum[:])
            F = small.tile([B, 4], F32)
            v.memset(F[:], 0.0)
            v.tensor_scalar_mul(F[:, 0:1], epb[:], 1.0 / (B * NB))
            v.tensor_scalar_mul(F[:, 1:2], enb[:], 1.0 / (B * (K - 1) * NB))
            v.tensor_copy(F[:, 2:3], contrib[:])
            out_ps = ps_one.tile([4, 1], F32, tag="o3")
            nc.tensor.matmul(out_ps[:], F[:], ones_sb[0:B, :], start=True, stop=True)
            out_sb = small.tile([4, 1], F32)
            a.copy(out_sb[:], out_ps[:])
            nc.sync.dma_start(out=out_d[:], in_=out_sb[:])

    nc.compile()
    return nc


_NC = None


def _get_nc():
    global _NC
    if _NC is None:
        _NC = _build()
    return _NC


def _make_in_maps(hazards, S, indiv, gene, path, cohort_bank, label, c):
    hazards = np.asarray(hazards, dtype=np.float32)
    S = np.asarray(S, dtype=np.float32)
    indiv = np.asarray(indiv, dtype=np.float32)
    gene = np.asarray(gene, dtype=np.float32)
    path = np.asarray(path, dtype=np.float32)
    cohort_bank = np.asarray(cohort_bank, dtype=np.float32)
    label = np.asarray(label)
    c = np.asarray(c)

    oh5 = np.zeros((B, K + 1), np.float32)
    oh5[np.arange(B), label] = 1.0
    oh5b = np.zeros((B, K + 1), np.float32)
    oh5b[np.arange(B), label + 1] = 1.0
    oh4 = oh5[:, :K].copy()
    spad = np.concatenate([np.ones((B, 1), np.float32), S], axis=1)
    cfs = np.stack([c.astype(np.float32), 1.0 - c.astype(np.float32)], axis=1)
    smallin = np.ascontiguousarray(np.concatenate([
        indiv.reshape(B, -1),
        gene.reshape(B, -1), path.reshape(B, -1),
        hazards, spad, oh5, oh5b, oh4, cfs,
    ], axis=1, dtype=np.float32))
    common = dict(smallin=smallin)
    bankf = cohort_bank.reshape(K, NB, 1024)
    in_maps = []
    for i in range(NCORES):
        shard = np.ascontiguousarray(
            bankf[:, i * NSH:(i + 1) * NSH, :]).reshape(ROWS, 1024)
        in_maps.append({**common, "bank": shard})
    return in_maps


_LAST_RESULTS = None  # stashed for test.py introspection


def kernel(hazards, S, indiv, gene, path, cohort_bank, label, c):
    global _LAST_RESULTS
    os.environ.setdefault("NEURON_RT_RESET_CORES", "1")
    nc = _get_nc()
    in_maps = _make_in_maps(hazards, S, indiv, gene, path, cohort_bank, label, c)
    trace = bool(int(os.environ.get("TRNK_TRACE", "0")))
    res = run_bass_kernel_spmd(nc, in_maps, core_ids=list(range(NCORES)),
                               trace=trace)
    _LAST_RESULTS = res
    outs = np.stack([r["out_vec"][:, 0] for r in res.results])  # [8, 4]
    ep = float(outs[:, 0].sum())
    en = float(outs[:, 1].sum())
    other = float(outs[:, 2].mean())
    loss = other - math.log((ep + EPS_COH) / (ep + en + EPS_COH))
    return np.float32(loss)



# revision 29
# speedup vs baseline: 1.0511x; 1.0185x over previous
"""Trainium2 Bass kernel for nn_Loss_factory_12429635355015.

Loss = NLLSurv + CohortLoss(intra + inter) over a [4, 8192, 4, 256] cohort bank.

Strategy (memory-bound, 8 NeuronCores):
  - Shard cohort_bank along the N (bank-entry) axis: each core streams its
    16 MiB shard once at HBM line rate (8 tiles x 2 MiB SWDGE cast-DMAs,
    f32 HBM -> bf16 SBUF); the small [B,*] tensors are replicated and ride
    in ONE merged DMA issued before the bank stream starts.
  - Per 512-entry tile (4 entries per partition), balanced across engines:
      DVE:  component-sum (3 bf16 adds); half the sum-of-squares (fused
            scalar_tensor_tensor); ONE broadcast normalize mul; one copy
      ACT:  other half of the sum-of-squares; rsqrt via exp(-.5 ln x); one
            copy; final e = exp(.5*sims) with accum_out row sums for free
      PE :  8x [128,128] transpose -> [c, n]; 2 matmuls vs l2-normed anchors
  - NLL + intra instruction emission is interleaved between tile emissions
    to fill pipeline slack; intra sims use raw dots scaled at the end by
    rsi_p*rsg_g on [B,8] (norm scales commute with abs/neg).
  - NLL + intra terms are computed on-device from host-encoded one-hots
    (index encoding only; all arithmetic on device).
  - Each core outputs [ep_partial, en_partial, nll+intra]; the host sums the
    two scalars across cores (the 'all-reduce two scalars' step) and applies
    the final -log((ep+eps)/(ep+en+eps)).
"""

import math
import os
import sys

import numpy as np

for _p in ("/opt/trn_rl_repo",):
    if _p not in sys.path and os.path.isdir(_p):
        sys.path.insert(0, _p)

import concourse.bacc as bacc
import concourse.tile as tile
from concourse import mybir
from concourse.bass_utils import run_bass_kernel_spmd

# Pin every activation to the one table set that contains all functions this
# kernel uses (Square/Ln/Exp/Copy/Abs/Identity). Without this, Bacc's
# first-match set selection alternates between sets (Ln lives outside the
# default exp set) and reloads the ACT tables ~1.3us per switch every tile.
_ACT_SET = "natural_log_exp_and_others"


def _pin_act_tables():
    import functools
    import concourse.hw_specs as hw_specs
    if getattr(hw_specs.get_activation_tables, "_pinned", False):
        return
    orig = hw_specs.get_activation_tables

    @functools.cache
    def pinned(arch):
        tabs = orig(arch)
        return {k: (v if k == _ACT_SET else set()) for k, v in tabs.items()}

    pinned._pinned = True
    hw_specs.get_activation_tables = pinned
    bacc.get_activation_tables = pinned


_pin_act_tables()

F32 = mybir.dt.float32
AF = mybir.ActivationFunctionType

# Problem constants (hardcoded per spec).
B = 64            # batch
K = 4             # n_cls
C = 256           # feature dim
NB = 8192         # bank entries per class (global)
NCORES = 8
NSH = NB // NCORES          # 1024 bank entries per class per core
ROWS = K * NSH              # 4096 rows of [4*256] per core
NT = 512                    # bank entries per tile (2 MiB)
TILES = ROWS // NT          # 8
TILES_PER_CLASS = NSH // NT # 2
EPG = NT // 128             # 4 entries per partition per tile
LN_HALF = math.log(0.5)
EPS_NLL = 1e-7
EPS_COH = 1e-8


def _build():
    nc = bacc.Bacc("TRN2", target_bir_lowering=False, debug=False,
                   enable_asserts=False, num_devices=NCORES)

    bank = nc.dram_tensor("bank", [ROWS, 1024], F32, kind="ExternalInput")
    # All small per-sample tensors ride in one [B, SMW] tensor -> ONE DMA at
    # startup instead of nine serialized ~620ns HWDGE launches.
    # Layout: indiv(1024) | gp(512) | haz(4) | spad(5) | ohy(5) | ohy1(5)
    #         | oh4(4) | cfs(2)
    SMW = 1024 + 512 + K + (K + 1) * 3 + K + 2
    smallin = nc.dram_tensor("smallin", [B, SMW], F32, kind="ExternalInput")

    out_d = nc.dram_tensor("out_vec", [4, 1], F32, kind="ExternalOutput")

    import ml_dtypes
    ident_d = nc.inline_tensor(np.eye(128, dtype=np.float32), "ident")
    ident_bf_d = nc.inline_tensor(np.eye(128, dtype=ml_dtypes.bfloat16), "ident_bf")
    ones_d = nc.inline_tensor(np.ones((128, 1), dtype=np.float32), "ones_col")

    v = nc.vector
    a = nc.scalar

    with tile.TileContext(nc) as tc:
        from contextlib import ExitStack
        with ExitStack() as ctx:
            const = ctx.enter_context(tc.tile_pool(name="const", bufs=1))
            small = ctx.enter_context(tc.tile_pool(name="small", bufs=1))
            tpool = ctx.enter_context(tc.tile_pool(name="T", bufs=6))
            spool = ctx.enter_context(tc.tile_pool(name="S", bufs=4))
            stpool = ctx.enter_context(tc.tile_pool(name="STsb", bufs=6))
            epool = ctx.enter_context(tc.tile_pool(name="esb", bufs=4))
            sqpool = ctx.enter_context(tc.tile_pool(name="sq", bufs=3))
            ps_st = ctx.enter_context(tc.tile_pool(name="ps_st", bufs=4, space="PSUM"))
            ps_p = ctx.enter_context(tc.tile_pool(name="ps_p", bufs=2, space="PSUM"))
            ps_one = ctx.enter_context(tc.tile_pool(name="ps_one", bufs=1, space="PSUM"))

            BF16 = mybir.dt.bfloat16
            # Issue the small-input DMA FIRST: once the bank stream saturates
            # HBM, a q1 transfer packet-interleaves with it and takes ~15us —
            # gating the whole DVE program (anchors come first in its stream).
            sm_sb = small.tile([B, SMW], F32)
            nc.sync.dma_start(out=sm_sb[:], in_=smallin[:])
            ident_sb = const.tile([128, 128], F32)
            nc.sync.dma_start(out=ident_sb[:], in_=ident_d[:])
            ident_bf = const.tile([128, 128], BF16)
            nc.sync.dma_start(out=ident_bf[:], in_=ident_bf_d[:])
            ones_sb = const.tile([128, 1], F32)
            nc.sync.dma_start(out=ones_sb[:], in_=ones_d[:])

            # ---------- anchors: A = l2norm(mean_j indiv[b,j,:]) ----------
            _o = [0]

            def _sl(w):
                s = _o[0]
                _o[0] += w
                return sm_sb[:, s:s + w]

            ind_sb = _sl(1024)
            gp_sb = _sl(512)
            haz_sb = _sl(K)
            spad_sb = _sl(K + 1)
            ohy_sb = _sl(K + 1)
            ohy1_sb = _sl(K + 1)
            oh4_sb = _sl(K)
            cfs_sb = _sl(2)
            iv = ind_sb.rearrange("p (j c) -> p j c", j=4)
            asum = small.tile([B, C], F32)
            atmp = small.tile([B, C], F32)
            v.tensor_add(asum[:], iv[:, 0, :], iv[:, 1, :])
            v.tensor_add(atmp[:], iv[:, 2, :], iv[:, 3, :])
            v.tensor_add(asum[:], asum[:], atmp[:])
            sqa = small.tile([B, C], F32)
            ssa = small.tile([B, 1], F32)
            a.activation(sqa[:], asum[:], AF.Square, accum_out=ssa[:])
            lna = small.tile([B, 1], F32)
            a.activation(lna[:], ssa[:], AF.Ln)
            rsa = small.tile([B, 1], F32)
            a.activation(rsa[:], lna[:], AF.Exp, scale=-0.5)
            v.tensor_scalar_mul(asum[:], asum[:], rsa[:])
            at_ps = ps_one.tile([128, 2, B], F32, tag="at")
            for h in range(2):
                nc.tensor.transpose(at_ps[:, h, :], asum[:, h * 128:(h + 1) * 128],
                                    ident_sb[0:B, 0:B])
            at_sb = const.tile([128, 2, B], BF16)
            a.copy(at_sb[:], at_ps[:])

            # ---------- side computation (NLL + intra), STT-fused ----------
            # Emitted in chunks BETWEEN tile emissions so the ~8us of small
            # DVE/ACT ops fill per-tile pipeline slack instead of extending
            # the head (pre-loop) or the drain (post-loop).
            BYP = mybir.AluOpType.bypass
            MULOP = mybir.AluOpType.mult
            sN = {}

            def _nll():
                t5 = small.tile([B, K + 1], F32)
                sy = small.tile([B, 1], F32)
                hy = small.tile([B, 1], F32)
                sy1 = small.tile([B, 1], F32)
                v.scalar_tensor_tensor(out=t5[:], in0=spad_sb[:], scalar=0.0,
                                       in1=ohy_sb[:], op0=BYP, op1=MULOP,
                                       accum_out=sy[:])
                v.scalar_tensor_tensor(out=t5[:, 0:K], in0=haz_sb[:], scalar=0.0,
                                       in1=ohy_sb[:, 0:K], op0=BYP, op1=MULOP,
                                       accum_out=hy[:])
                v.scalar_tensor_tensor(out=t5[:], in0=spad_sb[:], scalar=0.0,
                                       in1=ohy1_sb[:], op0=BYP, op1=MULOP,
                                       accum_out=sy1[:])
                for x in (sy, hy, sy1):
                    v.tensor_scalar_max(x[:], x[:], EPS_NLL)
                lsy = small.tile([B, 1], F32)
                lhy = small.tile([B, 1], F32)
                lsy1 = small.tile([B, 1], F32)
                a.activation(lsy[:], sy[:], AF.Ln)
                a.activation(lhy[:], hy[:], AF.Ln)
                a.activation(lsy1[:], sy1[:], AF.Ln)
                tu = small.tile([B, 1], F32)
                tcen = small.tile([B, 1], F32)
                negl = small.tile([B, 1], F32)
                v.tensor_add(tu[:], lsy[:], lhy[:])
                v.tensor_mul(tu[:], tu[:], cfs_sb[:, 1:2])      # *(1-cf)
                v.tensor_mul(tcen[:], lsy1[:], cfs_sb[:, 0:1])  # *cf
                v.tensor_add(negl[:], tu[:], tcen[:])           # = -neg_l per b
                sN["negl"] = negl

            def _intra_sos():
                ssqi = small.tile([B, 4], F32)
                ssqg = small.tile([B, 2], F32)
                prod = small.tile([B, C], F32, tag="iprod")
                for p in range(4):
                    a.activation(prod[:], ind_sb[:, p * C:(p + 1) * C],
                                 AF.Square, accum_out=ssqi[:, p:p + 1])
                for g in range(2):
                    a.activation(prod[:], gp_sb[:, g * C:(g + 1) * C],
                                 AF.Square, accum_out=ssqg[:, g:g + 1])
                rsi = small.tile([B, 4], F32)
                a.activation(rsi[:], ssqi[:], AF.Ln)
                a.activation(rsi[:], rsi[:], AF.Exp, scale=-0.5)
                rsg = small.tile([B, 2], F32)
                a.activation(rsg[:], ssqg[:], AF.Ln)
                a.activation(rsg[:], rsg[:], AF.Exp, scale=-0.5)
                sN["rsi"], sN["rsg"] = rsi, rsg

            def _intra_sims():
                # raw dots on the UN-normalized rows; the l2norm scales are
                # applied afterwards on the tiny [B,8] result (sim_hat =
                # raw_dot * rsi_p * rsg_g, and rsi/rsg > 0 commute with
                # abs/neg) -- saves six [B,256] normalize muls
                D = small.tile([B, 8], F32)
                prod = small.tile([B, C], F32, tag="iprod2")
                for p in range(4):
                    for g in range(2):
                        col = p * 2 + g
                        v.scalar_tensor_tensor(
                            out=prod[:], in0=ind_sb[:, p * C:(p + 1) * C],
                            scalar=0.0, in1=gp_sb[:, g * C:(g + 1) * C],
                            op0=BYP, op1=MULOP, accum_out=D[:, col:col + 1])
                sN["D"] = D

            def _intra_fin():
                Draw = sN["D"]
                rprod = small.tile([B, 4, 2], F32)
                v.tensor_mul(rprod[:],
                             sN["rsi"][:, :, None].broadcast_to([B, 4, 2]),
                             sN["rsg"][:, None, :].broadcast_to([B, 4, 2]))
                D = small.tile([B, 8], F32, tag="Dh")
                v.tensor_mul(D[:], Draw[:], rprod.rearrange("p a b -> p (a b)"))
                U = small.tile([B, 8], F32)
                a.activation(U[:], D[:], AF.Abs)
                # mask==1 entries (cols 0,1,4,7) use -sim instead of |sim|
                v.tensor_scalar_mul(U[:, 0:2], D[:, 0:2], -1.0)
                v.tensor_scalar_mul(U[:, 4:5], D[:, 4:5], -1.0)
                v.tensor_scalar_mul(U[:, 7:8], D[:, 7:8], -1.0)
                isum = small.tile([B, 1], F32)
                v.reduce_sum(isum[:], U[:], axis=mybir.AxisListType.X)
                # contrib_b = -negl/B + isum/(8B) + 1/B -> sums to nll+intra
                c1 = small.tile([B, 1], F32)
                c2 = small.tile([B, 1], F32)
                contrib = small.tile([B, 1], F32)
                v.tensor_scalar_mul(c1[:], sN["negl"][:], -1.0 / B)
                v.tensor_scalar_mul(c2[:], isum[:], 1.0 / (8 * B))
                v.tensor_add(contrib[:], c1[:], c2[:])
                v.tensor_scalar_add(contrib[:], contrib[:], 1.0 / B)
                sN["contrib"] = contrib

            side = {1: _nll, 2: _intra_sos, 3: _intra_sims, 4: _intra_fin}

            # ---------- main loop over bank tiles ----------
            # Per-tile exp-sums land in et_all columns straight from the ACT
            # accumulator; the epilogue folds tile pairs into per-class sums.
            et_all = small.tile([B, TILES], F32)
            for t in range(TILES):
                T_sb = tpool.tile([128, 4096], BF16)
                src = bank[t * NT:(t + 1) * NT, :].rearrange("(p e) x -> p e x", e=EPG)
                # SWDGE cast-DMA: f32 HBM -> bf16 SBUF at line rate
                nc.gpsimd.dma_start(out=T_sb.rearrange("p (e x) -> p e x", e=EPG),
                                    in_=src)
                Tv = T_sb.rearrange("p (e j c) -> p e j c", e=EPG, j=4)
                S_sb = spool.tile([128, 1024], BF16)
                Sv = S_sb.rearrange("p (e c) -> p e c", e=EPG)
                tmp = spool.tile([128, 1024], BF16, tag="tmp")
                tv = tmp.rearrange("p (e c) -> p e c", e=EPG)
                v.tensor_add(Sv[:], Tv[:, :, 0, :], Tv[:, :, 1, :])
                v.tensor_add(tv[:], Tv[:, :, 2, :], Tv[:, :, 3, :])
                v.tensor_add(Sv[:], Sv[:], tv[:])
                # per-entry sum-of-squares, split across ACT (Square+accum)
                # and DVE (fused square+row-sum) to balance the two engines
                ssum4 = spool.tile([128, EPG], F32, tag="ssum")
                sqscr = sqpool.tile([128, C], BF16)
                sqscrf = sqpool.tile([128, C], F32, tag="sqf")
                for e in range(2):
                    a.activation(sqscrf[:], Sv[:, e, :], AF.Square,
                                 accum_out=ssum4[:, e:e + 1])
                for e in range(2, EPG):
                    v.scalar_tensor_tensor(
                        out=sqscr[:], in0=Sv[:, e, :], scalar=0.0,
                        in1=Sv[:, e, :], op0=BYP, op1=MULOP,
                        accum_out=ssum4[:, e:e + 1])
                lh4 = spool.tile([128, EPG], F32, tag="lh4")
                a.activation(lh4[:], ssum4[:], AF.Ln)
                # rsqrt lands in bf16 so the fused normalize runs in 2x mode
                rh4 = spool.tile([128, EPG], BF16, tag="rh4")
                a.activation(rh4[:], lh4[:], AF.Exp, scale=-0.5)
                # normalize S rows in place: ONE broadcast mul over all 4
                # e-groups (per-entry scale broadcast along c)
                v.tensor_mul(Sv[:], Sv[:],
                             rh4[:, :, None].broadcast_to([128, EPG, C]))
                # transpose all 4 e-groups into [c, n=512] chunks (h = c-half)
                st_ps = [ps_st.tile([128, 512], BF16, name="stps", tag="stps")
                         for _ in range(2)]
                for e in range(EPG):
                    for h in range(2):
                        nc.tensor.transpose(
                            st_ps[h][:, e * 128:(e + 1) * 128],
                            S_sb[:, e * C + h * 128: e * C + (h + 1) * 128],
                            ident_bf[:])
                p_ps = ps_p.tile([B, 512], F32)
                for h in range(2):
                    st_sb = stpool.tile([128, 512], BF16)
                    a.copy(st_sb[:], st_ps[h][:])   # both on ACT: DVE paces
                    nc.tensor.matmul(p_ps[:], at_sb[:, h, :], st_sb[:],
                                     start=(h == 0), stop=(h == 1))
                e_sb = epool.tile([B, 512], BF16)
                a.activation(e_sb[:], p_ps[:], AF.Exp, scale=0.5,
                             accum_out=et_all[:, t:t + 1])
                if t in side:
                    side[t]()

            # ---------- epilogue: partial scalars ----------
            contrib = sN["contrib"]
            E_sb = small.tile([B, K], F32)
            eav = et_all.rearrange("p (k two) -> p k two", two=TILES_PER_CLASS)
            v.tensor_add(E_sb[:], eav[:, :, 0], eav[:, :, 1])
            t4b = small.tile([B, K], F32)
            epb = small.tile([B, 1], F32)
            rsum = small.tile([B, 1], F32)
            enb = small.tile([B, 1], F32)
            v.tensor_mul(t4b[:], E_sb[:], oh4_sb[:])
            v.reduce_sum(epb[:], t4b[:], axis=mybir.AxisListType.X)
            v.reduce_sum(rsum[:], E_sb[:], axis=mybir.AxisListType.X)
            v.tensor_scalar_mul(enb[:], epb[:], -1.0)
            v.tensor_add(enb[:], enb[:], rsum[:])
            F = small.tile([B, 4], F32)
            v.memset(F[:], 0.0)
            v.tensor_scalar_mul(F[:, 0:1], epb[:], 1.0 / (B * NB))
            v.tensor_scalar_mul(F[:, 1:2], enb[:], 1.0 / (B * (K - 1) * NB))
            v.tensor_copy(F[:, 2:3], contrib[:])
            out_ps = ps_one.tile([4, 1], F32, tag="o3")
            nc.tensor.matmul(out_ps[:], F[:], ones_sb[0:B, :], start=True, stop=True)
            out_sb = small.tile([4, 1], F32)
            a.copy(out_sb[:], out_ps[:])
            nc.sync.dma_start(out=out_d[:], in_=out_sb[:])

    nc.compile()
    return nc


_NC = None


def _get_nc():
    global _NC
    if _NC is None:
        _NC = _build()
    return _NC


def _make_in_maps(hazards, S, indiv, gene, path, cohort_bank, label, c):
    hazards = np.asarray(hazards, dtype=np.float32)
    S = np.asarray(S, dtype=np.float32)
    indiv = np.asarray(indiv, dtype=np.float32)
    gene = np.asarray(gene, dtype=np.float32)
    path = np.asarray(path, dtype=np.float32)
    cohort_bank = np.asarray(cohort_bank, dtype=np.float32)
    label = np.asarray(label)
    c = np.asarray(c)

    oh5 = np.zeros((B, K + 1), np.float32)
    oh5[np.arange(B), label] = 1.0
    oh5b = np.zeros((B, K + 1), np.float32)
    oh5b[np.arange(B), label + 1] = 1.0
    oh4 = oh5[:, :K].copy()
    spad = np.concatenate([np.ones((B, 1), np.float32), S], axis=1)
    cfs = np.stack([c.astype(np.float32), 1.0 - c.astype(np.float32)], axis=1)
    smallin = np.ascontiguousarray(np.concatenate([
        indiv.reshape(B, -1),
        gene.reshape(B, -1), path.reshape(B, -1),
        hazards, spad, oh5, oh5b, oh4, cfs,
    ], axis=1, dtype=np.float32))
    common = dict(smallin=smallin)
    bankf = cohort_bank.reshape(K, NB, 1024)
    in_maps = []
    for i in range(NCORES):
        shard = np.ascontiguousarray(
            bankf[:, i * NSH:(i + 1) * NSH, :]).reshape(ROWS, 1024)
        in_maps.append({**common, "bank": shard})
    return in_maps


_LAST_RESULTS = None  # stashed for test.py introspection


def kernel(hazards, S, indiv, gene, path, cohort_bank, label, c):
    global _LAST_RESULTS
    os.environ.setdefault("NEURON_RT_RESET_CORES", "1")
    nc = _get_nc()
    in_maps = _make_in_maps(hazards, S, indiv, gene, path, cohort_bank, label, c)
    trace = bool(int(os.environ.get("TRNK_TRACE", "0")))
    res = run_bass_kernel_spmd(nc, in_maps, core_ids=list(range(NCORES)),
                               trace=trace)
    _LAST_RESULTS = res
    outs = np.stack([r["out_vec"][:, 0] for r in res.results])  # [8, 4]
    ep = float(outs[:, 0].sum())
    en = float(outs[:, 1].sum())
    other = float(outs[:, 2].mean())
    loss = other - math.log((ep + EPS_COH) / (ep + en + EPS_COH))
    return np.float32(loss)



# revision 30
# speedup vs baseline: 1.0907x; 1.0377x over previous
"""Trainium2 Bass kernel for nn_Loss_factory_12429635355015.

Loss = NLLSurv + CohortLoss(intra + inter) over a [4, 8192, 4, 256] cohort bank.

Strategy (memory-bound, 8 NeuronCores):
  - Shard cohort_bank along the N (bank-entry) axis: each core streams its
    16 MiB shard once at HBM line rate (8 tiles x 2 MiB SWDGE cast-DMAs,
    f32 HBM -> bf16 SBUF); the small [B,*] tensors are replicated and ride
    in ONE merged DMA issued before the bank stream starts.
  - Per 512-entry tile (4 entries per partition), balanced across engines:
      DVE:  component-sum (3 bf16 adds); half the sum-of-squares (fused
            scalar_tensor_tensor); ONE broadcast normalize mul; one copy
      ACT:  other half of the sum-of-squares; rsqrt via exp(-.5 ln x); one
            copy; final e = exp(.5*sims) with accum_out row sums for free
      PE :  8x [128,128] transpose -> [c, n]; 2 matmuls vs l2-normed anchors
  - NLL + intra instruction emission is interleaved between tile emissions
    to fill pipeline slack; intra sims use raw dots scaled at the end by
    rsi_p*rsg_g on [B,8] (norm scales commute with abs/neg).
  - NLL + intra terms are computed on-device from host-encoded one-hots
    (index encoding only; all arithmetic on device).
  - Each core outputs [ep_partial, en_partial, nll+intra]; the host sums the
    two scalars across cores (the 'all-reduce two scalars' step) and applies
    the final -log((ep+eps)/(ep+en+eps)).
"""

import math
import os
import sys

import numpy as np

for _p in ("/opt/trn_rl_repo",):
    if _p not in sys.path and os.path.isdir(_p):
        sys.path.insert(0, _p)

import concourse.bacc as bacc
import concourse.tile as tile
from concourse import mybir
from concourse.bass_utils import run_bass_kernel_spmd

# Pin every activation to the one table set that contains all functions this
# kernel uses (Square/Ln/Exp/Copy/Abs/Identity). Without this, Bacc's
# first-match set selection alternates between sets (Ln lives outside the
# default exp set) and reloads the ACT tables ~1.3us per switch every tile.
_ACT_SET = "natural_log_exp_and_others"


def _pin_act_tables():
    import functools
    import concourse.hw_specs as hw_specs
    if getattr(hw_specs.get_activation_tables, "_pinned", False):
        return
    orig = hw_specs.get_activation_tables

    @functools.cache
    def pinned(arch):
        tabs = orig(arch)
        return {k: (v if k == _ACT_SET else set()) for k, v in tabs.items()}

    pinned._pinned = True
    hw_specs.get_activation_tables = pinned
    bacc.get_activation_tables = pinned


_pin_act_tables()

F32 = mybir.dt.float32
AF = mybir.ActivationFunctionType

# Problem constants (hardcoded per spec).
B = 64            # batch
K = 4             # n_cls
C = 256           # feature dim
NB = 8192         # bank entries per class (global)
NCORES = 8
NSH = NB // NCORES          # 1024 bank entries per class per core
ROWS = K * NSH              # 4096 rows of [4*256] per core
NT = 512                    # bank entries per tile (2 MiB)
TILES = ROWS // NT          # 8
TILES_PER_CLASS = NSH // NT # 2
EPG = NT // 128             # 4 entries per partition per tile
LN_HALF = math.log(0.5)
EPS_NLL = 1e-7
EPS_COH = 1e-8


def _build():
    nc = bacc.Bacc("TRN2", target_bir_lowering=False, debug=False,
                   enable_asserts=False, num_devices=NCORES)

    bank = nc.dram_tensor("bank", [ROWS, 1024], F32, kind="ExternalInput")
    # All small per-sample tensors ride in one [B, SMW] tensor -> ONE DMA at
    # startup instead of nine serialized ~620ns HWDGE launches.
    # Layout: indiv(1024) | gp(512) | haz(4) | spad(5) | ohy(5) | ohy1(5)
    #         | oh4(4) | cfs(2)
    SMW = 1024 + 512 + K + (K + 1) * 3 + K + 2
    smallin = nc.dram_tensor("smallin", [B, SMW], F32, kind="ExternalInput")

    out_d = nc.dram_tensor("out_vec", [4, 1], F32, kind="ExternalOutput")

    import ml_dtypes
    ident_d = nc.inline_tensor(np.eye(128, dtype=np.float32), "ident")
    ident_bf_d = nc.inline_tensor(np.eye(128, dtype=ml_dtypes.bfloat16), "ident_bf")
    ones_d = nc.inline_tensor(np.ones((128, 1), dtype=np.float32), "ones_col")

    v = nc.vector
    a = nc.scalar

    with tile.TileContext(nc) as tc:
        from contextlib import ExitStack
        with ExitStack() as ctx:
            const = ctx.enter_context(tc.tile_pool(name="const", bufs=1))
            small = ctx.enter_context(tc.tile_pool(name="small", bufs=1))
            tpool = ctx.enter_context(tc.tile_pool(name="T", bufs=6))
            spool = ctx.enter_context(tc.tile_pool(name="S", bufs=4))
            stpool = ctx.enter_context(tc.tile_pool(name="STsb", bufs=6))
            epool = ctx.enter_context(tc.tile_pool(name="esb", bufs=4))
            sqpool = ctx.enter_context(tc.tile_pool(name="sq", bufs=3))
            ps_st = ctx.enter_context(tc.tile_pool(name="ps_st", bufs=4, space="PSUM"))
            ps_p = ctx.enter_context(tc.tile_pool(name="ps_p", bufs=2, space="PSUM"))
            ps_one = ctx.enter_context(tc.tile_pool(name="ps_one", bufs=1, space="PSUM"))

            BF16 = mybir.dt.bfloat16
            # Issue the small-input DMA FIRST: once the bank stream saturates
            # HBM, a q1 transfer packet-interleaves with it and takes ~15us —
            # gating the whole DVE program (anchors come first in its stream).
            sm_sb = small.tile([B, SMW], F32)
            nc.sync.dma_start(out=sm_sb[:], in_=smallin[:])
            ident_sb = const.tile([128, 128], F32)
            nc.sync.dma_start(out=ident_sb[:], in_=ident_d[:])
            ident_bf = const.tile([128, 128], BF16)
            nc.sync.dma_start(out=ident_bf[:], in_=ident_bf_d[:])
            ones_sb = const.tile([128, 1], F32)
            nc.sync.dma_start(out=ones_sb[:], in_=ones_d[:])

            # ---------- anchors: A = l2norm(mean_j indiv[b,j,:]) ----------
            _o = [0]

            def _sl(w):
                s = _o[0]
                _o[0] += w
                return sm_sb[:, s:s + w]

            ind_sb = _sl(1024)
            gp_sb = _sl(512)
            haz_sb = _sl(K)
            spad_sb = _sl(K + 1)
            ohy_sb = _sl(K + 1)
            ohy1_sb = _sl(K + 1)
            oh4_sb = _sl(K)
            cfs_sb = _sl(2)
            iv = ind_sb.rearrange("p (j c) -> p j c", j=4)
            asum = small.tile([B, C], F32)
            atmp = small.tile([B, C], F32)
            v.tensor_add(asum[:], iv[:, 0, :], iv[:, 1, :])
            v.tensor_add(atmp[:], iv[:, 2, :], iv[:, 3, :])
            v.tensor_add(asum[:], asum[:], atmp[:])
            sqa = small.tile([B, C], F32)
            ssa = small.tile([B, 1], F32)
            a.activation(sqa[:], asum[:], AF.Square, accum_out=ssa[:])
            lna = small.tile([B, 1], F32)
            a.activation(lna[:], ssa[:], AF.Ln)
            rsa = small.tile([B, 1], F32)
            a.activation(rsa[:], lna[:], AF.Exp, scale=-0.5)
            v.tensor_scalar_mul(asum[:], asum[:], rsa[:])
            at_ps = ps_one.tile([128, 2, B], F32, tag="at")
            for h in range(2):
                nc.tensor.transpose(at_ps[:, h, :], asum[:, h * 128:(h + 1) * 128],
                                    ident_sb[0:B, 0:B])
            at_sb = const.tile([128, 2, B], BF16)
            a.copy(at_sb[:], at_ps[:])

            # ---------- side computation (NLL + intra), STT-fused ----------
            # Emitted in chunks BETWEEN tile emissions so the ~8us of small
            # DVE/ACT ops fill per-tile pipeline slack instead of extending
            # the head (pre-loop) or the drain (post-loop).
            BYP = mybir.AluOpType.bypass
            MULOP = mybir.AluOpType.mult
            sN = {}

            def _nll():
                t5 = small.tile([B, K + 1], F32)
                sy = small.tile([B, 1], F32)
                hy = small.tile([B, 1], F32)
                sy1 = small.tile([B, 1], F32)
                v.scalar_tensor_tensor(out=t5[:], in0=spad_sb[:], scalar=0.0,
                                       in1=ohy_sb[:], op0=BYP, op1=MULOP,
                                       accum_out=sy[:])
                v.scalar_tensor_tensor(out=t5[:, 0:K], in0=haz_sb[:], scalar=0.0,
                                       in1=ohy_sb[:, 0:K], op0=BYP, op1=MULOP,
                                       accum_out=hy[:])
                v.scalar_tensor_tensor(out=t5[:], in0=spad_sb[:], scalar=0.0,
                                       in1=ohy1_sb[:], op0=BYP, op1=MULOP,
                                       accum_out=sy1[:])
                for x in (sy, hy, sy1):
                    v.tensor_scalar_max(x[:], x[:], EPS_NLL)
                lsy = small.tile([B, 1], F32)
                lhy = small.tile([B, 1], F32)
                lsy1 = small.tile([B, 1], F32)
                a.activation(lsy[:], sy[:], AF.Ln)
                a.activation(lhy[:], hy[:], AF.Ln)
                a.activation(lsy1[:], sy1[:], AF.Ln)
                tu = small.tile([B, 1], F32)
                tcen = small.tile([B, 1], F32)
                negl = small.tile([B, 1], F32)
                v.tensor_add(tu[:], lsy[:], lhy[:])
                v.tensor_mul(tu[:], tu[:], cfs_sb[:, 1:2])      # *(1-cf)
                v.tensor_mul(tcen[:], lsy1[:], cfs_sb[:, 0:1])  # *cf
                v.tensor_add(negl[:], tu[:], tcen[:])           # = -neg_l per b
                sN["negl"] = negl

            def _intra_sos():
                ssqi = small.tile([B, 4], F32)
                ssqg = small.tile([B, 2], F32)
                prod = small.tile([B, C], F32, tag="iprod")
                for p in range(4):
                    v.scalar_tensor_tensor(
                        out=prod[:], in0=ind_sb[:, p * C:(p + 1) * C], scalar=0.0,
                        in1=ind_sb[:, p * C:(p + 1) * C], op0=BYP, op1=MULOP,
                        accum_out=ssqi[:, p:p + 1])
                for g in range(2):
                    v.scalar_tensor_tensor(
                        out=prod[:], in0=gp_sb[:, g * C:(g + 1) * C], scalar=0.0,
                        in1=gp_sb[:, g * C:(g + 1) * C], op0=BYP, op1=MULOP,
                        accum_out=ssqg[:, g:g + 1])
                rsi = small.tile([B, 4], F32)
                a.activation(rsi[:], ssqi[:], AF.Ln)
                a.activation(rsi[:], rsi[:], AF.Exp, scale=-0.5)
                rsg = small.tile([B, 2], F32)
                a.activation(rsg[:], ssqg[:], AF.Ln)
                a.activation(rsg[:], rsg[:], AF.Exp, scale=-0.5)
                sN["rsi"], sN["rsg"] = rsi, rsg

            def _intra_sims():
                # raw dots on the UN-normalized rows; the l2norm scales are
                # applied afterwards on the tiny [B,8] result (sim_hat =
                # raw_dot * rsi_p * rsg_g, and rsi/rsg > 0 commute with
                # abs/neg) -- saves six [B,256] normalize muls
                D = small.tile([B, 8], F32)
                prod = small.tile([B, C], F32, tag="iprod2")
                for p in range(4):
                    for g in range(2):
                        col = p * 2 + g
                        v.scalar_tensor_tensor(
                            out=prod[:], in0=ind_sb[:, p * C:(p + 1) * C],
                            scalar=0.0, in1=gp_sb[:, g * C:(g + 1) * C],
                            op0=BYP, op1=MULOP, accum_out=D[:, col:col + 1])
                sN["D"] = D

            def _intra_fin():
                Draw = sN["D"]
                rprod = small.tile([B, 4, 2], F32)
                v.tensor_mul(rprod[:],
                             sN["rsi"][:, :, None].broadcast_to([B, 4, 2]),
                             sN["rsg"][:, None, :].broadcast_to([B, 4, 2]))
                D = small.tile([B, 8], F32, tag="Dh")
                v.tensor_mul(D[:], Draw[:], rprod.rearrange("p a b -> p (a b)"))
                U = small.tile([B, 8], F32)
                a.activation(U[:], D[:], AF.Abs)
                # mask==1 entries (cols 0,1,4,7) use -sim instead of |sim|
                v.tensor_scalar_mul(U[:, 0:2], D[:, 0:2], -1.0)
                v.tensor_scalar_mul(U[:, 4:5], D[:, 4:5], -1.0)
                v.tensor_scalar_mul(U[:, 7:8], D[:, 7:8], -1.0)
                isum = small.tile([B, 1], F32)
                v.reduce_sum(isum[:], U[:], axis=mybir.AxisListType.X)
                # contrib_b = -negl/B + isum/(8B) + 1/B -> sums to nll+intra
                c1 = small.tile([B, 1], F32)
                c2 = small.tile([B, 1], F32)
                contrib = small.tile([B, 1], F32)
                v.tensor_scalar_mul(c1[:], sN["negl"][:], -1.0 / B)
                v.tensor_scalar_mul(c2[:], isum[:], 1.0 / (8 * B))
                v.tensor_add(contrib[:], c1[:], c2[:])
                v.tensor_scalar_add(contrib[:], contrib[:], 1.0 / B)
                sN["contrib"] = contrib

            side = {1: _nll, 2: _intra_sos, 3: _intra_sims, 4: _intra_fin}

            # ---------- main loop over bank tiles ----------
            # Per-tile exp-sums land in et_all columns straight from the ACT
            # accumulator; the epilogue folds tile pairs into per-class sums.
            et_all = small.tile([B, TILES], F32)
            for t in range(TILES):
                T_sb = tpool.tile([128, 4096], BF16)
                src = bank[t * NT:(t + 1) * NT, :].rearrange("(p e) x -> p e x", e=EPG)
                # SWDGE cast-DMA: f32 HBM -> bf16 SBUF at line rate
                nc.gpsimd.dma_start(out=T_sb.rearrange("p (e x) -> p e x", e=EPG),
                                    in_=src)
                Tv = T_sb.rearrange("p (e j c) -> p e j c", e=EPG, j=4)
                S_sb = spool.tile([128, 1024], BF16)
                Sv = S_sb.rearrange("p (e c) -> p e c", e=EPG)
                tmp = spool.tile([128, 1024], BF16, tag="tmp")
                tv = tmp.rearrange("p (e c) -> p e c", e=EPG)
                v.tensor_add(Sv[:], Tv[:, :, 0, :], Tv[:, :, 1, :])
                v.tensor_add(tv[:], Tv[:, :, 2, :], Tv[:, :, 3, :])
                v.tensor_add(Sv[:], Sv[:], tv[:])
                # per-entry sum-of-squares, split across ACT (Square+accum)
                # and DVE (fused square+row-sum) to balance the two engines
                ssum4 = spool.tile([128, EPG], F32, tag="ssum")
                sqscr = sqpool.tile([128, C], BF16)
                sqscrf = sqpool.tile([128, C], F32, tag="sqf")
                for e in range(2):
                    a.activation(sqscrf[:], Sv[:, e, :], AF.Square,
                                 accum_out=ssum4[:, e:e + 1])
                for e in range(2, EPG):
                    v.scalar_tensor_tensor(
                        out=sqscr[:], in0=Sv[:, e, :], scalar=0.0,
                        in1=Sv[:, e, :], op0=BYP, op1=MULOP,
                        accum_out=ssum4[:, e:e + 1])
                lh4 = spool.tile([128, EPG], F32, tag="lh4")
                a.activation(lh4[:], ssum4[:], AF.Ln)
                # rsqrt lands in bf16 so the fused normalize runs in 2x mode
                rh4 = spool.tile([128, EPG], BF16, tag="rh4")
                a.activation(rh4[:], lh4[:], AF.Exp, scale=-0.5)
                # normalize S rows in place: ONE broadcast mul over all 4
                # e-groups (per-entry scale broadcast along c)
                v.tensor_mul(Sv[:], Sv[:],
                             rh4[:, :, None].broadcast_to([128, EPG, C]))
                # transpose all 4 e-groups into [c, n=512] chunks (h = c-half)
                st_ps = [ps_st.tile([128, 512], BF16, name="stps", tag="stps")
                         for _ in range(2)]
                for e in range(EPG):
                    for h in range(2):
                        nc.tensor.transpose(
                            st_ps[h][:, e * 128:(e + 1) * 128],
                            S_sb[:, e * C + h * 128: e * C + (h + 1) * 128],
                            ident_bf[:])
                p_ps = ps_p.tile([B, 512], F32)
                for h in range(2):
                    st_sb = stpool.tile([128, 512], BF16)
                    if h == 0:
                        a.copy(st_sb[:], st_ps[h][:])       # ACT has slack
                    else:
                        v.tensor_copy(st_sb[:], st_ps[h][:])
                    nc.tensor.matmul(p_ps[:], at_sb[:, h, :], st_sb[:],
                                     start=(h == 0), stop=(h == 1))
                e_sb = epool.tile([B, 512], BF16)
                a.activation(e_sb[:], p_ps[:], AF.Exp, scale=0.5,
                             accum_out=et_all[:, t:t + 1])
                if t in side:
                    side[t]()

            # ---------- epilogue: partial scalars ----------
            contrib = sN["contrib"]
            E_sb = small.tile([B, K], F32)
            eav = et_all.rearrange("p (k two) -> p k two", two=TILES_PER_CLASS)
            v.tensor_add(E_sb[:], eav[:, :, 0], eav[:, :, 1])
            t4b = small.tile([B, K], F32)
            epb = small.tile([B, 1], F32)
            rsum = small.tile([B, 1], F32)
            enb = small.tile([B, 1], F32)
            v.tensor_mul(t4b[:], E_sb[:], oh4_sb[:])
            v.reduce_sum(epb[:], t4b[:], axis=mybir.AxisListType.X)
            v.reduce_sum(rsum[:], E_sb[:], axis=mybir.AxisListType.X)
            v.tensor_scalar_mul(enb[:], epb[:], -1.0)
            v.tensor_add(enb[:], enb[:], rsum[:])
            F = small.tile([B, 4], F32)
            v.memset(F[:], 0.0)
            v.tensor_scalar_mul(F[:, 0:1], epb[:], 1.0 / (B * NB))
            v.tensor_scalar_mul(F[:, 1:2], enb[:], 1.0 / (B * (K - 1) * NB))
            v.tensor_copy(F[:, 2:3], contrib[:])
            out_ps = ps_one.tile([4, 1], F32, tag="o3")
            nc.tensor.matmul(out_ps[:], F[:], ones_sb[0:B, :], start=True, stop=True)
            out_sb = small.tile([4, 1], F32)
            a.copy(out_sb[:], out_ps[:])
            nc.sync.dma_start(out=out_d[:], in_=out_sb[:])

    nc.compile()
    return nc


_NC = None


def _get_nc():
    global _NC
    if _NC is None:
        _NC = _build()
    return _NC


def _make_in_maps(hazards, S, indiv, gene, path, cohort_bank, label, c):
    hazards = np.asarray(hazards, dtype=np.float32)
    S = np.asarray(S, dtype=np.float32)
    indiv = np.asarray(indiv, dtype=np.float32)
    gene = np.asarray(gene, dtype=np.float32)
    path = np.asarray(path, dtype=np.float32)
    cohort_bank = np.asarray(cohort_bank, dtype=np.float32)
    label = np.asarray(label)
    c = np.asarray(c)

    oh5 = np.zeros((B, K + 1), np.float32)
    oh5[np.arange(B), label] = 1.0
    oh5b = np.zeros((B, K + 1), np.float32)
    oh5b[np.arange(B), label + 1] = 1.0
    oh4 = oh5[:, :K].copy()
    spad = np.concatenate([np.ones((B, 1), np.float32), S], axis=1)
    cfs = np.stack([c.astype(np.float32), 1.0 - c.astype(np.float32)], axis=1)
    smallin = np.ascontiguousarray(np.concatenate([
        indiv.reshape(B, -1),
        gene.reshape(B, -1), path.reshape(B, -1),
        hazards, spad, oh5, oh5b, oh4, cfs,
    ], axis=1, dtype=np.float32))
    common = dict(smallin=smallin)
    bankf = cohort_bank.reshape(K, NB, 1024)
    in_maps = []
    for i in range(NCORES):
        shard = np.ascontiguousarray(
            bankf[:, i * NSH:(i + 1) * NSH, :]).reshape(ROWS, 1024)
        in_maps.append({**common, "bank": shard})
    return in_maps


_LAST_RESULTS = None  # stashed for test.py introspection


def kernel(hazards, S, indiv, gene, path, cohort_bank, label, c):
    global _LAST_RESULTS
    os.environ.setdefault("NEURON_RT_RESET_CORES", "1")
    nc = _get_nc()
    in_maps = _make_in_maps(hazards, S, indiv, gene, path, cohort_bank, label, c)
    trace = bool(int(os.environ.get("TRNK_TRACE", "0")))
    res = run_bass_kernel_spmd(nc, in_maps, core_ids=list(range(NCORES)),
                               trace=trace)
    _LAST_RESULTS = res
    outs = np.stack([r["out_vec"][:, 0] for r in res.results])  # [8, 4]
    ep = float(outs[:, 0].sum())
    en = float(outs[:, 1].sum())
    other = float(outs[:, 2].mean())
    loss = other - math.log((ep + EPS_COH) / (ep + en + EPS_COH))
    return np.float32(loss)

